# revision 5
# baseline (speedup 1.0000x reference)
"""Causal self-attention (B=4, T=2048, D=1024, H=16) on 8 Trainium2 NeuronCores.

Sharding: batch x head-half. Core c handles batch b = c//2 and heads
hh..hh+7 where hh = 8*(c%2)  (tensor-parallel split of w_qkv output dim and
w_o input dim). Each core produces a partial o_proj output [2048, 1024];
the host sums the two partials per batch (the 2-way all-reduce).

Per-core kernel (all matmuls bf16, fp32 PSUM accumulate). Head pairs are
fused into [128, 1024] two-bank PSUM tiles so ACT/DVE/DMA process both
heads with one instruction.

v2: software-pipelined schedule. The attention chunk stream
(scores -> exp -> PV) is ACT(exp)-bound per chunk (~1147ns ACT vs ~640ns
PE), so the PE queue is emitted as S(c+1) before P(c) (depth-1 pipeline)
and QKV/V projection matmuls are chopped into 8-matmul half-units that are
injected into the stream whenever the modeled ACT-minus-PE deficit exceeds
one unit. o_proj runs in blocks at q-tile seams, reusing the PV psum slots.
Causal masking multiplies run on gpsimd to keep DVE free for psum->sbuf
copies. x/w input tiles are double-buffered so the next rep's input DMA
overlaps the current rep's tail.
"""
import numpy as np
import ml_dtypes

B, T, D, H = 4, 2048, 1024, 16
DK = D // H          # 64
HPC = 8              # heads per core
NCORES = 8
NQT = T // 512       # 4
NTT = T // 128       # 16

_cache = {}


def _emit(nc, tc, pools, dram, opts=()):
    import concourse.mybir as mybir

    bf16 = mybir.dt.bfloat16
    f32 = mybir.dt.float32
    Exp = mybir.ActivationFunctionType.Exp
    cst, big, work, norm, stp, ps = pools
    xt_d, wq_d, wo_d, out_d, masks = dram

    xts = [big.tile([128, T], bf16, tag=f"xt{dc}", name=f"xts{dc}")
           for dc in range(8)]
    wqs = [big.tile([128, 1536], bf16, tag=f"wq{dc}", name=f"wqs{dc}")
           for dc in range(8)]
    wos = [big.tile([128, D], bf16, tag=f"wo{pr}", name=f"wos{pr}")
           for pr in range(4)]
    qk = big.tile([128, 8, T], bf16, tag="qk")      # [2hd, ec(q0-3,k4-7), t]
    vt = big.tile([128, NTT, HPC, DK + 1], bf16, tag="vt")
    ob = big.tile([128, 4, T], bf16, tag="ob")

    # input DMA spread over three launch queues, low dc chunks first on each
    for dc in range(0, 8, 2):
        nc.sync.dma_start(xts[dc][:], xt_d[dc])
        nc.gpsimd.dma_start(xts[dc + 1][:], xt_d[dc + 1])
        nc.scalar.dma_start(wqs[dc][:], wq_d[dc])
        nc.scalar.dma_start(wqs[dc + 1][:], wq_d[dc + 1])
    for pr in range(4):
        nc.scalar.dma_start(wos[pr][:], wo_d[pr])

    nc.gpsimd.memset(vt[:, :, :, DK], 1.0)

    # ---- projection half-units (8 matmuls + one copy, emitted whole) ----
    def h_qk(ec, w):
        # q (ec 0-3) or k (ec 4-7) rows for token window w -> qk[:, ec, :]
        pst = ps.tile([128, 512], f32, tag="s", name=f"qkps{ec}_{w}")
        for dc in range(8):
            nc.tensor.matmul(
                pst[:], wqs[dc][:, ec * 128:(ec + 1) * 128],
                xts[dc][:, w * 512:(w + 1) * 512],
                start=(dc == 0), stop=(dc == 7))
        nc.vector.tensor_copy(qk[:, ec, w * 512:(w + 1) * 512], pst[:])

    def h_v(tt):
        pst = ps.tile([128, 512], f32, tag="s", name=f"vps{tt}")
        for dc in range(8):
            nc.tensor.matmul(
                pst[:], xts[dc][:, tt * 128:(tt + 1) * 128],
                wqs[dc][:, 1024:1536],
                start=(dc == 0), stop=(dc == 7))
        nc.vector.tensor_copy(
            vt[:, tt, :, 0:DK],
            pst[:].rearrange("p (h d) -> p h d", d=DK))

    def h_o(qt, tt):
        po = ps.tile([128, 1024], f32, tag="pv", name=f"po{tt}")
        for eh in range(2):
            for pr in range(4):
                nc.tensor.matmul(
                    po[:, eh * 512:(eh + 1) * 512],
                    ob[:, pr, tt * 128:(tt + 1) * 128],
                    wos[pr][:, eh * 512:(eh + 1) * 512],
                    start=(pr == 0), stop=(pr == 3))
        ot = work.tile([128, 1024], f32, tag="ot")
        nc.vector.tensor_copy(ot[:], po[:])
        nc.gpsimd.dma_start(out_d[tt * 128:(tt + 1) * 128, :], ot[:])

    # ---- filler queue: dependency-ordered; groups flush a prefix, the
    # deficit counter pulls the rest between attention chunks ----
    fillers = []
    fidx = [0]

    def pull_one():
        if fidx[0] < len(fillers):
            fillers[fidx[0]]()
            fidx[0] += 1
            return True
        return False

    def flush_to(n):
        while fidx[0] < n:
            pull_one()

    # preamble units (emitted directly): deps of G(0,0) scores + first PVs
    pre = [lambda: h_qk(4, 0), lambda: h_qk(0, 0),
           lambda: h_v(0), lambda: h_v(1)]

    def fq(ec, w):
        return lambda: h_qk(ec, w)

    def fv(tt):
        return lambda: h_v(tt)

    # group order is qt-major: G(0,0..3), O0, G(1,0..3), O1, ...
    # each group's new deps form a prefix of the remaining filler queue
    gdep = {}
    for qt in range(4):
        for pr in range(4):
            if qt == 0 and pr == 0:
                fillers += [fv(2), fv(3)]
            elif pr == 0:
                fillers += [fq(4, qt), fq(0, qt)]
                fillers += [fv(tt) for tt in range(4 * qt, 4 * qt + 4)]
            else:
                fillers += [fq(4 + pr, qt), fq(pr, qt)]
            gdep[(qt, pr)] = len(fillers)

    # ---- attention chunk stream ----
    def s_chunk(qt, pr, kc, sts):
        i = kc - 4 * qt
        lo = max(i, 0) * 128
        sp = ps.tile([128, 1024], f32, tag="s", name=f"sp{qt}_{pr}_{kc}")
        nc.tensor.matmul(
            sp[:, lo:512], qk[0:64, 4 + pr, kc * 128:(kc + 1) * 128],
            qk[0:64, pr, qt * 512 + lo:(qt + 1) * 512],
            start=True, stop=True, tile_position=(0, 0))
        nc.tensor.matmul(
            sp[:, 512 + lo:1024],
            qk[64:128, 4 + pr, kc * 128:(kc + 1) * 128],
            qk[64:128, pr, qt * 512 + lo:(qt + 1) * 512],
            start=True, stop=True, tile_position=(64, 0))
        st = stp.tile([128, 1024], bf16, tag="st", name=f"st{qt}_{pr}_{kc}")
        sp3 = sp[:].rearrange("p (h q) -> p h q", h=2)[:, :, lo:]
        st3 = st[:].rearrange("p (h q) -> p h q", h=2)[:, :, lo:]
        nc.scalar.activation(st3, sp3, Exp, scale=0.125)
        if i >= 0:      # diagonal chunk: mask the triangular blocks
            nc.gpsimd.tensor_mul(
                st[:, lo:lo + 128], st[:, lo:lo + 128], masks[:])
            nc.gpsimd.tensor_mul(
                st[:, 512 + lo:512 + lo + 128],
                st[:, 512 + lo:512 + lo + 128], masks[:])
        sts[kc] = st

    def p_chunk(qt, pr, kc, nkc, pvp, sts):
        lo = max(kc - 4 * qt, 0) * 128
        st = sts[kc]
        nc.tensor.matmul(
            pvp[:, lo:512], vt[:, kc, 2 * pr, :], st[:, lo:512],
            start=(kc == 0), stop=(kc == nkc - 1))
        nc.tensor.matmul(
            pvp[:, 512 + lo:1024], vt[:, kc, 2 * pr + 1, :],
            st[:, 512 + lo:1024],
            start=(kc == 0), stop=(kc == nkc - 1))

    def n_group(qt, pr, pvp):
        # normalize both heads: ob[:, pr, qt] = pv[0:64] * (1/rowsum)
        # (custom-DVE ops ignore the input AP partition base, so stage the
        # sums row at partition 0 first)
        sd = norm.tile([1, 1024], f32, tag="sd")
        nc.vector.tensor_copy(sd[:], pvp[64:65, :])
        nc.vector.reciprocal_approx_fast(sd[:], sd[:])
        rb = norm.tile([64, 1024], f32, tag="rb")
        nc.gpsimd.partition_broadcast(rb[:], sd[:])
        for hh in range(2):
            nc.vector.tensor_mul(
                ob[64 * hh:64 * hh + 64, pr, qt * 512:(qt + 1) * 512],
                pvp[0:64, hh * 512:(hh + 1) * 512],
                rb[:, hh * 512:(hh + 1) * 512])

    # modeled per-chunk engine costs (ns) drive filler injection
    FILL_NS = 1800.0        # one 8-matmul half-unit on PE

    deficit = [0.0]

    def maybe_fill():
        while deficit[0] >= FILL_NS:
            if not pull_one():
                return
            deficit[0] -= FILL_NS

    for fn in pre:
        fn()

    for qt in range(4):
        for pr in range(4):
            flush_to(gdep[(qt, pr)])
            nkc = 4 * qt + 4
            pvp = ps.tile([65, 1024], f32, tag="pv", name=f"pvp{qt}_{pr}")
            sts = {}
            for kc in range(nkc):
                lo = max(kc - 4 * qt, 0) * 128
                s_chunk(qt, pr, kc, sts)
                if kc > 0:
                    p_chunk(qt, pr, kc - 1, nkc, pvp, sts)
                deficit[0] += ((1024 - 2 * lo + 352) / 1.2
                               - (3 * (512 - lo) / 2.4 + 120.0))
                maybe_fill()
            # cover exp(last) latency with one filler before the final PV
            pull_one()
            p_chunk(qt, pr, nkc - 1, nkc, pvp, sts)
            n_group(qt, pr, pvp)
        pull_one()      # cover the normalize chain before o_proj reads ob
        for tt in range(4 * qt, 4 * qt + 4):
            h_o(qt, tt)
    flush_to(len(fillers))


def _build(reps=1, opts=()):
    import concourse.mybir as mybir
    import concourse.tile as tile
    from concourse import bacc

    bf16 = mybir.dt.bfloat16
    f32 = mybir.dt.float32

    nc = bacc.Bacc("TRN2", target_bir_lowering=False, debug=False,
                   num_devices=NCORES)
    xt_d = nc.dram_tensor("xt", [8, 128, T], bf16, kind="ExternalInput")
    wq_d = nc.dram_tensor("wq", [8, 128, 1536], bf16, kind="ExternalInput")
    wo_d = nc.dram_tensor("wo", [4, 128, D], bf16, kind="ExternalInput")
    out_d = nc.dram_tensor("out", [T, D], f32, kind="ExternalOutput")

    with tile.TileContext(nc) as tc:
        with (
            tc.tile_pool(name="cst", bufs=1) as cst,
            tc.tile_pool(name="big", bufs=1) as big,
            tc.tile_pool(name="work", bufs=2) as work,
            tc.tile_pool(name="norm", bufs=2) as norm,
            tc.tile_pool(name="stp", bufs=6) as stp,
            tc.tile_pool(name="ps", bufs=2, space="PSUM") as ps,
        ):
            # static causal mask for the 128x128 diagonal blocks:
            # masks[p, q] = 1 if q >= p else 0
            masks = cst.tile([128, 128], bf16)
            nc.gpsimd.memset(masks[:], 1.0)
            nc.gpsimd.affine_select(
                out=masks[:], in_=masks[:],
                compare_op=mybir.AluOpType.is_ge, fill=0.0,
                base=0, channel_multiplier=-1, pattern=[[1, 128]],
            )
            pools = (cst, big, work, norm, stp, ps)
            dram = (xt_d, wq_d, wo_d, out_d, masks)
            if reps == 1:
                _emit(nc, tc, pools, dram, opts)
            else:
                with tc.For_i(0, reps, 1):
                    _emit(nc, tc, pools, dram, opts)

    nc.compile()
    return nc


def prep_inputs(x, w_qkv, w_o):
    """Host-side shard + layout prep. Returns in_maps for cores 0..7."""
    bf = ml_dtypes.bfloat16
    in_maps = []
    for c in range(NCORES):
        b, hh = c // 2, HPC * (c % 2)
        qrows = w_qkv[hh * DK:(hh + HPC) * DK]                    # [512, 1024]
        krows = w_qkv[D + hh * DK:D + (hh + HPC) * DK]
        vrows = w_qkv[2 * D + hh * DK:2 * D + (hh + HPC) * DK]
        wqt = np.concatenate([qrows, krows, vrows], 0).T          # [1024, 1536]
        in_maps.append({
            "xt": np.ascontiguousarray(x[b].T).astype(bf).reshape(8, 128, T),
            "wq": wqt.astype(bf).reshape(8, 128, 1536),
            "wo": np.ascontiguousarray(w_o[:, hh * DK:(hh + HPC) * DK].T)
                    .astype(bf).reshape(4, 128, D),
        })
    return in_maps


def get_nc(reps=1, opts=()):
    key = ("nc", reps, tuple(opts))
    if key not in _cache:
        _cache[key] = _build(reps, tuple(opts))
    return _cache[key]


def kernel(x, w_qkv, w_o):
    from concourse.bass_utils import run_bass_kernel_spmd

    nc = get_nc()
    in_maps = prep_inputs(np.asarray(x, dtype=np.float32),
                          np.asarray(w_qkv, dtype=np.float32),
                          np.asarray(w_o, dtype=np.float32))
    try:
        res = run_bass_kernel_spmd(nc, in_maps, core_ids=list(range(NCORES)))
    except Exception:
        # transient device faults (e.g. NRT_EXEC_UNIT_UNRECOVERABLE) have
        # been observed once on an otherwise-correct build; retry once
        res = run_bass_kernel_spmd(nc, in_maps, core_ids=list(range(NCORES)))
    out = np.empty((B, T, D), np.float32)
    for b in range(B):
        out[b] = res.results[2 * b]["out"] + res.results[2 * b + 1]["out"]
    return out


# revision 6
# speedup vs baseline: 1.4129x; 1.4129x over previous
"""Causal self-attention (B=4, T=2048, D=1024, H=16) on 8 Trainium2 NeuronCores.

Sharding: batch x head-half. Core c handles batch b = c//2 and heads
hh..hh+7 where hh = 8*(c%2)  (tensor-parallel split of w_qkv output dim and
w_o input dim). Each core produces a partial o_proj output [2048, 1024];
the host sums the two partials per batch (the 2-way all-reduce).

Per-core kernel (all matmuls bf16, fp32 PSUM accumulate). Head pairs are
fused into [128, 1024] two-bank PSUM tiles so ACT/DVE/DMA process both
heads with one instruction.

v2: software-pipelined schedule. The attention chunk stream
(scores -> exp -> PV) is ACT(exp)-bound per chunk (~1147ns ACT vs ~640ns
PE), so the PE queue is emitted as S(c+1) before P(c) (depth-1 pipeline)
and QKV/V projection matmuls are chopped into 8-matmul half-units that are
injected into the stream whenever the modeled ACT-minus-PE deficit exceeds
one unit. o_proj runs in blocks at q-tile seams, reusing the PV psum slots.
Causal masking multiplies run on gpsimd to keep DVE free for psum->sbuf
copies. x/w input tiles are double-buffered so the next rep's input DMA
overlaps the current rep's tail.
"""
import numpy as np
import ml_dtypes

B, T, D, H = 4, 2048, 1024, 16
DK = D // H          # 64
HPC = 8              # heads per core
NCORES = 8
NQT = T // 512       # 4
NTT = T // 128       # 16

_cache = {}


def _emit(nc, tc, pools, dram, opts=()):
    import concourse.mybir as mybir

    bf16 = mybir.dt.bfloat16
    f32 = mybir.dt.float32
    Exp = mybir.ActivationFunctionType.Exp
    cst, big, work, norm, stp, ps = pools
    xt_d, wq_d, wo_d, out_d, masks = dram

    xts = [big.tile([128, T], bf16, tag=f"xt{dc}", name=f"xts{dc}")
           for dc in range(8)]
    wqs = [big.tile([128, 1536], bf16, tag=f"wq{dc}", name=f"wqs{dc}")
           for dc in range(8)]
    wos = [big.tile([128, D], bf16, tag=f"wo{pr}", name=f"wos{pr}")
           for pr in range(4)]
    qk = big.tile([128, 8, T], bf16, tag="qk")      # [2hd, ec(q0-3,k4-7), t]
    vt = big.tile([128, NTT, HPC, DK + 1], bf16, tag="vt")
    ob = big.tile([128, 4, T], bf16, tag="ob")

    # input DMA spread over three launch queues, low dc chunks first on each
    for dc in range(0, 8, 2):
        nc.sync.dma_start(xts[dc][:], xt_d[dc])
        nc.sync.dma_start(xts[dc + 1][:], xt_d[dc + 1])
        nc.scalar.dma_start(wqs[dc][:], wq_d[dc])
        nc.scalar.dma_start(wqs[dc + 1][:], wq_d[dc + 1])
    for pr in range(4):
        nc.scalar.dma_start(wos[pr][:], wo_d[pr])

    nc.gpsimd.memset(vt[:, :, :, DK], 1.0)

    # ---- projection half-units (8 matmuls + one copy, emitted whole) ----
    def h_qk(ec, w):
        # q (ec 0-3) or k (ec 4-7) rows for token window w -> qk[:, ec, :]
        pst = ps.tile([128, 512], f32, tag="s", name=f"qkps{ec}_{w}")
        for dc in range(8):
            nc.tensor.matmul(
                pst[:], wqs[dc][:, ec * 128:(ec + 1) * 128],
                xts[dc][:, w * 512:(w + 1) * 512],
                start=(dc == 0), stop=(dc == 7))
        nc.vector.tensor_copy(qk[:, ec, w * 512:(w + 1) * 512], pst[:])

    def h_v(tt):
        pst = ps.tile([128, 512], f32, tag="s", name=f"vps{tt}")
        for dc in range(8):
            nc.tensor.matmul(
                pst[:], xts[dc][:, tt * 128:(tt + 1) * 128],
                wqs[dc][:, 1024:1536],
                start=(dc == 0), stop=(dc == 7))
        nc.vector.tensor_copy(
            vt[:, tt, :, 0:DK],
            pst[:].rearrange("p (h d) -> p h d", d=DK))

    def h_o(qt, tt):
        po = ps.tile([128, 1024], f32, tag="pv", name=f"po{tt}")
        for eh in range(2):
            for pr in range(4):
                nc.tensor.matmul(
                    po[:, eh * 512:(eh + 1) * 512],
                    ob[:, pr, tt * 128:(tt + 1) * 128],
                    wos[pr][:, eh * 512:(eh + 1) * 512],
                    start=(pr == 0), stop=(pr == 3))
        ot = work.tile([128, 1024], f32, tag="ot")
        nc.vector.tensor_copy(ot[:], po[:])
        nc.gpsimd.dma_start(out_d[tt * 128:(tt + 1) * 128, :], ot[:])

    # ---- filler queue: dependency-ordered; groups flush a prefix, the
    # deficit counter pulls the rest between attention chunks ----
    fillers = []
    fidx = [0]

    def pull_one():
        if fidx[0] < len(fillers):
            fillers[fidx[0]]()
            fidx[0] += 1
            return True
        return False

    def flush_to(n):
        while fidx[0] < n:
            pull_one()

    # preamble units (emitted directly): deps of G(0,0) scores + first PVs
    pre = [lambda: h_qk(4, 0), lambda: h_qk(0, 0),
           lambda: h_v(0), lambda: h_v(1)]

    def fq(ec, w):
        return lambda: h_qk(ec, w)

    def fv(tt):
        return lambda: h_v(tt)

    # group order is qt-major: G(0,0..3), O0, G(1,0..3), O1, ...
    # each group's new deps form a prefix of the remaining filler queue
    gdep = {}
    for qt in range(4):
        for pr in range(4):
            if qt == 0 and pr == 0:
                fillers += [fv(2), fv(3)]
            elif pr == 0:
                fillers += [fq(4, qt), fq(0, qt)]
                fillers += [fv(tt) for tt in range(4 * qt, 4 * qt + 4)]
            else:
                fillers += [fq(4 + pr, qt), fq(pr, qt)]
            gdep[(qt, pr)] = len(fillers)

    # ---- attention chunk stream ----
    def s_chunk(qt, pr, kc, sts):
        i = kc - 4 * qt
        lo = max(i, 0) * 128
        sp = ps.tile([128, 1024], f32, tag="s", name=f"sp{qt}_{pr}_{kc}")
        nc.tensor.matmul(
            sp[:, lo:512], qk[0:64, 4 + pr, kc * 128:(kc + 1) * 128],
            qk[0:64, pr, qt * 512 + lo:(qt + 1) * 512],
            start=True, stop=True, tile_position=(0, 0))
        nc.tensor.matmul(
            sp[:, 512 + lo:1024],
            qk[64:128, 4 + pr, kc * 128:(kc + 1) * 128],
            qk[64:128, pr, qt * 512 + lo:(qt + 1) * 512],
            start=True, stop=True, tile_position=(64, 0))
        st = stp.tile([128, 1024], bf16, tag="st", name=f"st{qt}_{pr}_{kc}")
        sp3 = sp[:].rearrange("p (h q) -> p h q", h=2)[:, :, lo:]
        st3 = st[:].rearrange("p (h q) -> p h q", h=2)[:, :, lo:]
        nc.scalar.activation(st3, sp3, Exp, scale=0.125)
        if i >= 0:      # diagonal chunk: mask the triangular blocks
            nc.vector.tensor_mul(
                st[:, lo:lo + 128], st[:, lo:lo + 128], masks[:])
            nc.vector.tensor_mul(
                st[:, 512 + lo:512 + lo + 128],
                st[:, 512 + lo:512 + lo + 128], masks[:])
        sts[kc] = st

    def p_chunk(qt, pr, kc, nkc, pvp, sts):
        lo = max(kc - 4 * qt, 0) * 128
        st = sts[kc]
        nc.tensor.matmul(
            pvp[:, lo:512], vt[:, kc, 2 * pr, :], st[:, lo:512],
            start=(kc == 0), stop=(kc == nkc - 1))
        nc.tensor.matmul(
            pvp[:, 512 + lo:1024], vt[:, kc, 2 * pr + 1, :],
            st[:, 512 + lo:1024],
            start=(kc == 0), stop=(kc == nkc - 1))

    def n_group(qt, pr, pvp):
        # normalize both heads: ob[:, pr, qt] = pv[0:64] * (1/rowsum)
        # (custom-DVE ops ignore the input AP partition base, so stage the
        # sums row at partition 0 first)
        sd = norm.tile([1, 1024], f32, tag="sd")
        nc.vector.tensor_copy(sd[:], pvp[64:65, :])
        nc.vector.reciprocal_approx_fast(sd[:], sd[:])
        rb = norm.tile([64, 1024], f32, tag="rb")
        nc.gpsimd.partition_broadcast(rb[:], sd[:])
        for hh in range(2):
            nc.vector.tensor_mul(
                ob[64 * hh:64 * hh + 64, pr, qt * 512:(qt + 1) * 512],
                pvp[0:64, hh * 512:(hh + 1) * 512],
                rb[:, hh * 512:(hh + 1) * 512])

    # modeled per-chunk engine costs (ns) drive filler injection
    FILL_NS = 1800.0        # one 8-matmul half-unit on PE

    deficit = [0.0]

    def maybe_fill():
        while deficit[0] >= FILL_NS:
            if not pull_one():
                return
            deficit[0] -= FILL_NS

    for fn in pre:
        fn()

    for qt in range(4):
        for pr in range(4):
            flush_to(gdep[(qt, pr)])
            nkc = 4 * qt + 4
            pvp = ps.tile([65, 1024], f32, tag="pv", name=f"pvp{qt}_{pr}")
            sts = {}
            for kc in range(nkc):
                lo = max(kc - 4 * qt, 0) * 128
                s_chunk(qt, pr, kc, sts)
                if kc > 0:
                    p_chunk(qt, pr, kc - 1, nkc, pvp, sts)
                deficit[0] += ((1024 - 2 * lo + 352) / 1.2
                               - (3 * (512 - lo) / 2.4 + 120.0))
                maybe_fill()
            # cover exp(last) latency with one filler before the final PV
            pull_one()
            p_chunk(qt, pr, nkc - 1, nkc, pvp, sts)
            n_group(qt, pr, pvp)
        pull_one()      # cover the normalize chain before o_proj reads ob
        for tt in range(4 * qt, 4 * qt + 4):
            h_o(qt, tt)
    flush_to(len(fillers))


def _build(reps=1, opts=()):
    import concourse.mybir as mybir
    import concourse.tile as tile
    from concourse import bacc

    bf16 = mybir.dt.bfloat16
    f32 = mybir.dt.float32

    nc = bacc.Bacc("TRN2", target_bir_lowering=False, debug=False,
                   num_devices=NCORES)
    xt_d = nc.dram_tensor("xt", [8, 128, T], bf16, kind="ExternalInput")
    wq_d = nc.dram_tensor("wq", [8, 128, 1536], bf16, kind="ExternalInput")
    wo_d = nc.dram_tensor("wo", [4, 128, D], bf16, kind="ExternalInput")
    out_d = nc.dram_tensor("out", [T, D], f32, kind="ExternalOutput")

    with tile.TileContext(nc) as tc:
        with (
            tc.tile_pool(name="cst", bufs=1) as cst,
            tc.tile_pool(name="big", bufs=1) as big,
            tc.tile_pool(name="work", bufs=2) as work,
            tc.tile_pool(name="norm", bufs=2) as norm,
            tc.tile_pool(name="stp", bufs=6) as stp,
            tc.tile_pool(name="ps", bufs=2, space="PSUM") as ps,
        ):
            # static causal mask for the 128x128 diagonal blocks:
            # masks[p, q] = 1 if q >= p else 0
            masks = cst.tile([128, 128], bf16)
            nc.gpsimd.memset(masks[:], 1.0)
            nc.gpsimd.affine_select(
                out=masks[:], in_=masks[:],
                compare_op=mybir.AluOpType.is_ge, fill=0.0,
                base=0, channel_multiplier=-1, pattern=[[1, 128]],
            )
            pools = (cst, big, work, norm, stp, ps)
            dram = (xt_d, wq_d, wo_d, out_d, masks)
            if reps == 1:
                _emit(nc, tc, pools, dram, opts)
            else:
                with tc.For_i(0, reps, 1):
                    _emit(nc, tc, pools, dram, opts)

    nc.compile()
    return nc


def prep_inputs(x, w_qkv, w_o):
    """Host-side shard + layout prep. Returns in_maps for cores 0..7."""
    bf = ml_dtypes.bfloat16
    in_maps = []
    for c in range(NCORES):
        b, hh = c // 2, HPC * (c % 2)
        qrows = w_qkv[hh * DK:(hh + HPC) * DK]                    # [512, 1024]
        krows = w_qkv[D + hh * DK:D + (hh + HPC) * DK]
        vrows = w_qkv[2 * D + hh * DK:2 * D + (hh + HPC) * DK]
        wqt = np.concatenate([qrows, krows, vrows], 0).T          # [1024, 1536]
        in_maps.append({
            "xt": np.ascontiguousarray(x[b].T).astype(bf).reshape(8, 128, T),
            "wq": wqt.astype(bf).reshape(8, 128, 1536),
            "wo": np.ascontiguousarray(w_o[:, hh * DK:(hh + HPC) * DK].T)
                    .astype(bf).reshape(4, 128, D),
        })
    return in_maps


def get_nc(reps=1, opts=()):
    key = ("nc", reps, tuple(opts))
    if key not in _cache:
        _cache[key] = _build(reps, tuple(opts))
    return _cache[key]


def kernel(x, w_qkv, w_o):
    from concourse.bass_utils import run_bass_kernel_spmd

    nc = get_nc()
    in_maps = prep_inputs(np.asarray(x, dtype=np.float32),
                          np.asarray(w_qkv, dtype=np.float32),
                          np.asarray(w_o, dtype=np.float32))
    try:
        res = run_bass_kernel_spmd(nc, in_maps, core_ids=list(range(NCORES)))
    except Exception:
        # transient device faults (e.g. NRT_EXEC_UNIT_UNRECOVERABLE) have
        # been observed once on an otherwise-correct build; retry once
        res = run_bass_kernel_spmd(nc, in_maps, core_ids=list(range(NCORES)))
    out = np.empty((B, T, D), np.float32)
    for b in range(B):
        out[b] = res.results[2 * b]["out"] + res.results[2 * b + 1]["out"]
    return out


# revision 7
# speedup vs baseline: 1.7060x; 1.2074x over previous
"""Causal self-attention (B=4, T=2048, D=1024, H=16) on 8 Trainium2 NeuronCores.

Sharding: batch x head-half. Core c handles batch b = c//2 and heads
hh..hh+7 where hh = 8*(c%2)  (tensor-parallel split of w_qkv output dim and
w_o input dim). Each core produces a partial o_proj output [2048, 1024];
the host sums the two partials per batch (the 2-way all-reduce).

Per-core kernel (all matmuls bf16, fp32 PSUM accumulate). Head pairs are
fused into [128, 1024] two-bank PSUM tiles so ACT/DVE/DMA process both
heads with one instruction.

v2: software-pipelined schedule. The attention chunk stream
(scores -> exp -> PV) is ACT(exp)-bound per chunk (~1147ns ACT vs ~640ns
PE), so the PE queue is emitted as S(c+1) before P(c) (depth-1 pipeline)
and QKV/V projection matmuls are chopped into 8-matmul half-units that are
injected into the stream whenever the modeled ACT-minus-PE deficit exceeds
one unit. o_proj runs in blocks at q-tile seams, reusing the PV psum slots.
Causal masking multiplies run on gpsimd to keep DVE free for psum->sbuf
copies. x/w input tiles are double-buffered so the next rep's input DMA
overlaps the current rep's tail.
"""
import numpy as np
import ml_dtypes

B, T, D, H = 4, 2048, 1024, 16
DK = D // H          # 64
HPC = 8              # heads per core
NCORES = 8
NQT = T // 512       # 4
NTT = T // 128       # 16

_cache = {}


def _emit(nc, tc, pools, dram, opts=()):
    import concourse.mybir as mybir

    bf16 = mybir.dt.bfloat16
    f32 = mybir.dt.float32
    Exp = mybir.ActivationFunctionType.Exp
    cst, big, work, norm, stp, ps = pools
    xt_d, wq_d, wo_d, out_d, masks = dram

    xts = [big.tile([128, T], bf16, tag=f"xt{dc}", name=f"xts{dc}")
           for dc in range(8)]
    wqs = [big.tile([128, 1536], bf16, tag=f"wq{dc}", name=f"wqs{dc}")
           for dc in range(8)]
    wos = [big.tile([128, D], bf16, tag=f"wo{pr}", name=f"wos{pr}")
           for pr in range(4)]
    qk = big.tile([128, 8, T], bf16, tag="qk")      # [2hd, ec(q0-3,k4-7), t]
    vt = big.tile([128, NTT, HPC, DK + 1], bf16, tag="vt")
    ob = big.tile([128, 4, T], bf16, tag="ob")

    # input DMA spread over three launch queues, low dc chunks first on each
    for dc in range(0, 8, 2):
        nc.sync.dma_start(xts[dc][:], xt_d[dc])
        nc.sync.dma_start(xts[dc + 1][:], xt_d[dc + 1])
        nc.scalar.dma_start(wqs[dc][:], wq_d[dc])
        nc.scalar.dma_start(wqs[dc + 1][:], wq_d[dc + 1])
    for pr in range(4):
        nc.scalar.dma_start(wos[pr][:], wo_d[pr])

    nc.gpsimd.memset(vt[:, :, :, DK], 1.0)

    # ---- projection half-units (8 matmuls + one copy, emitted whole) ----
    def h_qk(ec, w):
        # q (ec 0-3) or k (ec 4-7) rows for token window w -> qk[:, ec, :]
        pst = ps.tile([128, 512], f32, tag="s", name=f"qkps{ec}_{w}")
        for dc in range(8):
            nc.tensor.matmul(
                pst[:], wqs[dc][:, ec * 128:(ec + 1) * 128],
                xts[dc][:, w * 512:(w + 1) * 512],
                start=(dc == 0), stop=(dc == 7))
        nc.vector.tensor_copy(qk[:, ec, w * 512:(w + 1) * 512], pst[:])

    def h_v(tt):
        pst = ps.tile([128, 512], f32, tag="s", name=f"vps{tt}")
        for dc in range(8):
            nc.tensor.matmul(
                pst[:], xts[dc][:, tt * 128:(tt + 1) * 128],
                wqs[dc][:, 1024:1536],
                start=(dc == 0), stop=(dc == 7))
        nc.vector.tensor_copy(
            vt[:, tt, :, 0:DK],
            pst[:].rearrange("p (h d) -> p h d", d=DK))

    def h_o(qt, tt):
        po = ps.tile([128, 1024], f32, tag="pv", name=f"po{tt}")
        for eh in range(2):
            for pr in range(4):
                nc.tensor.matmul(
                    po[:, eh * 512:(eh + 1) * 512],
                    ob[:, pr, tt * 128:(tt + 1) * 128],
                    wos[pr][:, eh * 512:(eh + 1) * 512],
                    start=(pr == 0), stop=(pr == 3))
        ot = work.tile([128, 1024], f32, tag="ot")
        nc.vector.tensor_copy(ot[:], po[:])
        nc.gpsimd.dma_start(out_d[tt * 128:(tt + 1) * 128, :], ot[:])

    # ---- filler queue: dependency-ordered; groups flush a prefix, the
    # deficit counter pulls the rest between attention chunks ----
    fillers = []
    fidx = [0]

    def pull_one():
        if fidx[0] < len(fillers):
            fillers[fidx[0]]()
            fidx[0] += 1
            return True
        return False

    def flush_to(n):
        while fidx[0] < n:
            pull_one()

    # preamble units (emitted directly): deps of G(0,0) scores + first PVs
    pre = [lambda: h_qk(4, 0), lambda: h_qk(0, 0),
           lambda: h_v(0), lambda: h_v(1)]

    def fq(ec, w):
        return lambda: h_qk(ec, w)

    def fv(tt):
        return lambda: h_v(tt)

    # group order is qt-major: G(0,0..3), O0, G(1,0..3), O1, ...
    # each group's new deps form a prefix of the remaining filler queue
    gdep = {}
    for qt in range(4):
        for pr in range(4):
            if qt == 0 and pr == 0:
                fillers += [fv(2), fv(3)]
            elif pr == 0:
                fillers += [fq(4, qt), fq(0, qt)]
                fillers += [fv(tt) for tt in range(4 * qt, 4 * qt + 4)]
            else:
                fillers += [fq(4 + pr, qt), fq(pr, qt)]
            gdep[(qt, pr)] = len(fillers)

    # ---- attention chunk stream ----
    def s_chunk(qt, pr, kc, sts):
        i = kc - 4 * qt
        lo = max(i, 0) * 128
        sp = ps.tile([128, 1024], f32, tag="s", name=f"sp{qt}_{pr}_{kc}")
        nc.tensor.matmul(
            sp[:, lo:512], qk[0:64, 4 + pr, kc * 128:(kc + 1) * 128],
            qk[0:64, pr, qt * 512 + lo:(qt + 1) * 512],
            start=True, stop=True, tile_position=(0, 0))
        nc.tensor.matmul(
            sp[:, 512 + lo:1024],
            qk[64:128, 4 + pr, kc * 128:(kc + 1) * 128],
            qk[64:128, pr, qt * 512 + lo:(qt + 1) * 512],
            start=True, stop=True, tile_position=(64, 0))
        st = stp.tile([128, 1024], bf16, tag="st", name=f"st{qt}_{pr}_{kc}")
        if "noexp" in opts:
            nc.vector.tensor_copy(st[0:1, 0:2], sp[0:1, 0:2])
            sts[kc] = st
            return
        sp3 = sp[:].rearrange("p (h q) -> p h q", h=2)[:, :, lo:]
        st3 = st[:].rearrange("p (h q) -> p h q", h=2)[:, :, lo:]
        nc.scalar.activation(st3, sp3, Exp, scale=0.125)
        if i >= 0:      # diagonal chunk: mask the triangular blocks
            nc.vector.tensor_mul(
                st[:, lo:lo + 128], st[:, lo:lo + 128], masks[:])
            nc.vector.tensor_mul(
                st[:, 512 + lo:512 + lo + 128],
                st[:, 512 + lo:512 + lo + 128], masks[:])
        sts[kc] = st

    def p_chunk(qt, pr, kc, nkc, pvp, sts):
        lo = max(kc - 4 * qt, 0) * 128
        st = sts[kc]
        nc.tensor.matmul(
            pvp[:, lo:512], vt[:, kc, 2 * pr, :], st[:, lo:512],
            start=(kc == 0), stop=(kc == nkc - 1))
        nc.tensor.matmul(
            pvp[:, 512 + lo:1024], vt[:, kc, 2 * pr + 1, :],
            st[:, 512 + lo:1024],
            start=(kc == 0), stop=(kc == nkc - 1))

    def n_group(qt, pr, pvp):
        # normalize both heads: ob[:, pr, qt] = pv[0:64] * (1/rowsum)
        # (custom-DVE ops ignore the input AP partition base, so stage the
        # sums row at partition 0 first)
        sd = norm.tile([1, 1024], f32, tag="sd")
        nc.vector.tensor_copy(sd[:], pvp[64:65, :])
        nc.vector.reciprocal_approx_fast(sd[:], sd[:])
        rb = norm.tile([64, 1024], f32, tag="rb")
        nc.gpsimd.partition_broadcast(rb[:], sd[:])
        for hh in range(2):
            nc.vector.tensor_mul(
                ob[64 * hh:64 * hh + 64, pr, qt * 512:(qt + 1) * 512],
                pvp[0:64, hh * 512:(hh + 1) * 512],
                rb[:, hh * 512:(hh + 1) * 512])

    # modeled per-chunk engine costs (ns) drive filler injection
    FILL_NS = 1800.0        # one 8-matmul half-unit on PE

    deficit = [0.0]

    def maybe_fill():
        while deficit[0] >= FILL_NS:
            if not pull_one():
                return
            deficit[0] -= FILL_NS

    for fn in pre:
        fn()

    if "noattn" in opts:
        flush_to(len(fillers))
        for qt in range(4):
            for tt in range(4 * qt, 4 * qt + 4):
                h_o(qt, tt)
        return
    for qt in range(4):
        for pr in range(4):
            flush_to(gdep[(qt, pr)])
            nkc = 4 * qt + 4
            pvp = ps.tile([65, 1024], f32, tag="pv", name=f"pvp{qt}_{pr}")
            sts = {}
            for kc in range(nkc):
                lo = max(kc - 4 * qt, 0) * 128
                s_chunk(qt, pr, kc, sts)
                if kc > 0:
                    p_chunk(qt, pr, kc - 1, nkc, pvp, sts)
                deficit[0] += ((1024 - 2 * lo + 352) / 1.2
                               - (3 * (512 - lo) / 2.4 + 120.0))
                maybe_fill()
            # cover exp(last) latency with one filler before the final PV
            pull_one()
            p_chunk(qt, pr, nkc - 1, nkc, pvp, sts)
            n_group(qt, pr, pvp)
        pull_one()      # cover the normalize chain before o_proj reads ob
        for tt in range(4 * qt, 4 * qt + 4):
            h_o(qt, tt)
    flush_to(len(fillers))


def _build(reps=1, opts=()):
    import concourse.mybir as mybir
    import concourse.tile as tile
    from concourse import bacc

    bf16 = mybir.dt.bfloat16
    f32 = mybir.dt.float32

    nc = bacc.Bacc("TRN2", target_bir_lowering=False, debug=False,
                   num_devices=NCORES)
    xt_d = nc.dram_tensor("xt", [8, 128, T], bf16, kind="ExternalInput")
    wq_d = nc.dram_tensor("wq", [8, 128, 1536], bf16, kind="ExternalInput")
    wo_d = nc.dram_tensor("wo", [4, 128, D], bf16, kind="ExternalInput")
    out_d = nc.dram_tensor("out", [T, D], f32, kind="ExternalOutput")

    with tile.TileContext(nc) as tc:
        with (
            tc.tile_pool(name="cst", bufs=1) as cst,
            tc.tile_pool(name="big", bufs=1) as big,
            tc.tile_pool(name="work", bufs=2) as work,
            tc.tile_pool(name="norm", bufs=2) as norm,
            tc.tile_pool(name="stp", bufs=6) as stp,
            tc.tile_pool(name="ps", bufs=2, space="PSUM") as ps,
        ):
            # static causal mask for the 128x128 diagonal blocks:
            # masks[p, q] = 1 if q >= p else 0
            masks = cst.tile([128, 128], bf16)
            nc.gpsimd.memset(masks[:], 1.0)
            nc.gpsimd.affine_select(
                out=masks[:], in_=masks[:],
                compare_op=mybir.AluOpType.is_ge, fill=0.0,
                base=0, channel_multiplier=-1, pattern=[[1, 128]],
            )
            pools = (cst, big, work, norm, stp, ps)
            dram = (xt_d, wq_d, wo_d, out_d, masks)
            if reps == 1:
                _emit(nc, tc, pools, dram, opts)
            else:
                with tc.For_i(0, reps, 1):
                    _emit(nc, tc, pools, dram, opts)

    nc.compile()
    return nc


def prep_inputs(x, w_qkv, w_o):
    """Host-side shard + layout prep. Returns in_maps for cores 0..7."""
    bf = ml_dtypes.bfloat16
    in_maps = []
    for c in range(NCORES):
        b, hh = c // 2, HPC * (c % 2)
        qrows = w_qkv[hh * DK:(hh + HPC) * DK]                    # [512, 1024]
        krows = w_qkv[D + hh * DK:D + (hh + HPC) * DK]
        vrows = w_qkv[2 * D + hh * DK:2 * D + (hh + HPC) * DK]
        wqt = np.concatenate([qrows, krows, vrows], 0).T          # [1024, 1536]
        in_maps.append({
            "xt": np.ascontiguousarray(x[b].T).astype(bf).reshape(8, 128, T),
            "wq": wqt.astype(bf).reshape(8, 128, 1536),
            "wo": np.ascontiguousarray(w_o[:, hh * DK:(hh + HPC) * DK].T)
                    .astype(bf).reshape(4, 128, D),
        })
    return in_maps


def get_nc(reps=1, opts=()):
    key = ("nc", reps, tuple(opts))
    if key not in _cache:
        _cache[key] = _build(reps, tuple(opts))
    return _cache[key]


def kernel(x, w_qkv, w_o):
    from concourse.bass_utils import run_bass_kernel_spmd

    nc = get_nc()
    in_maps = prep_inputs(np.asarray(x, dtype=np.float32),
                          np.asarray(w_qkv, dtype=np.float32),
                          np.asarray(w_o, dtype=np.float32))
    try:
        res = run_bass_kernel_spmd(nc, in_maps, core_ids=list(range(NCORES)))
    except Exception:
        # transient device faults (e.g. NRT_EXEC_UNIT_UNRECOVERABLE) have
        # been observed once on an otherwise-correct build; retry once
        res = run_bass_kernel_spmd(nc, in_maps, core_ids=list(range(NCORES)))
    out = np.empty((B, T, D), np.float32)
    for b in range(B):
        out[b] = res.results[2 * b]["out"] + res.results[2 * b + 1]["out"]
    return out


# revision 9
# speedup vs baseline: 6.3817x; 3.7407x over previous
"""Causal self-attention (B=4, T=2048, D=1024, H=16) on 8 Trainium2 NeuronCores.

Sharding: batch x head-half. Core c handles batch b = c//2 and heads
hh..hh+7 where hh = 8*(c%2)  (tensor-parallel split of w_qkv output dim and
w_o input dim). Each core produces a partial o_proj output [2048, 1024];
the host sums the two partials per batch (the 2-way all-reduce).

Per-core kernel (all matmuls bf16, fp32 PSUM accumulate). Head pairs are
fused into [128, 1024] two-bank PSUM tiles so ACT/DVE/DMA process both
heads with one instruction.

v2: software-pipelined schedule. The attention chunk stream
(scores -> exp -> PV) is ACT(exp)-bound per chunk (~1147ns ACT vs ~640ns
PE), so the PE queue is emitted as S(c+1) before P(c) (depth-1 pipeline)
and QKV/V projection matmuls are chopped into 8-matmul half-units that are
injected into the stream whenever the modeled ACT-minus-PE deficit exceeds
one unit. o_proj runs in blocks at q-tile seams, reusing the PV psum slots.
Causal masking multiplies run on gpsimd to keep DVE free for psum->sbuf
copies. x/w input tiles are double-buffered so the next rep's input DMA
overlaps the current rep's tail.
"""
import numpy as np
import ml_dtypes

B, T, D, H = 4, 2048, 1024, 16
DK = D // H          # 64
HPC = 8              # heads per core
NCORES = 8
NQT = T // 512       # 4
NTT = T // 128       # 16

_cache = {}


def _emit(nc, tc, pools, dram, opts=()):
    import concourse.mybir as mybir

    bf16 = mybir.dt.bfloat16
    f32 = mybir.dt.float32
    Exp = mybir.ActivationFunctionType.Exp
    cst, big, work, norm, stp, ps = pools
    xt_d, wq_d, wo_d, out_d, masks = dram

    xts = [big.tile([128, T], bf16, tag=f"xt{dc}", name=f"xts{dc}")
           for dc in range(8)]
    wqs = [big.tile([128, 1536], bf16, tag=f"wq{dc}", name=f"wqs{dc}", bufs=2)
           for dc in range(8)]
    wos = [big.tile([128, D], bf16, tag=f"wo{pr}", name=f"wos{pr}")
           for pr in range(4)]
    qk = big.tile([128, 8, T], bf16, tag="qk")      # [2hd, ec(q0-3,k4-7), t]
    vt = big.tile([128, NTT, HPC, DK + 1], bf16, tag="vt")
    ob = big.tile([128, 4, T], bf16, tag="ob")

    # input DMA spread over three launch queues, low dc chunks first on each
    for dc in range(0, 8, 2):
        nc.sync.dma_start(xts[dc][:], xt_d[dc])
        nc.sync.dma_start(xts[dc + 1][:], xt_d[dc + 1])
        nc.scalar.dma_start(wqs[dc][:], wq_d[dc])
        nc.scalar.dma_start(wqs[dc + 1][:], wq_d[dc + 1])
    for pr in range(4):
        nc.scalar.dma_start(wos[pr][:], wo_d[pr])

    nc.gpsimd.memset(vt[:, :, :, 0], 1.0)

    # ---- projection half-units (8 matmuls + one copy, emitted whole) ----
    def h_qk(ec, w):
        # q (ec 0-3) or k (ec 4-7) rows for token window w -> qk[:, ec, :]
        pst = ps.tile([128, 512], f32, tag="s", name=f"qkps{ec}_{w}")
        for dc in range(8):
            nc.tensor.matmul(
                pst[:], wqs[dc][:, ec * 128:(ec + 1) * 128],
                xts[dc][:, w * 512:(w + 1) * 512],
                start=(dc == 0), stop=(dc == 7))
        nc.vector.tensor_copy(qk[:, ec, w * 512:(w + 1) * 512], pst[:])

    def h_v(tt):
        pst = ps.tile([128, 512], f32, tag="s", name=f"vps{tt}")
        for dc in range(8):
            nc.tensor.matmul(
                pst[:], xts[dc][:, tt * 128:(tt + 1) * 128],
                wqs[dc][:, 1024:1536],
                start=(dc == 0), stop=(dc == 7))
        nc.vector.tensor_copy(
            vt[:, tt, :, 1:DK + 1],
            pst[:].rearrange("p (h d) -> p h d", d=DK))

    def h_o(qt, tt):
        po = ps.tile([128, 1024], f32, tag="pv", name=f"po{tt}")
        for eh in range(2):
            for pr in range(4):
                nc.tensor.matmul(
                    po[:, eh * 512:(eh + 1) * 512],
                    ob[:, pr, tt * 128:(tt + 1) * 128],
                    wos[pr][:, eh * 512:(eh + 1) * 512],
                    start=(pr == 0), stop=(pr == 3))
        ot = work.tile([128, 1024], bf16, tag="ot")
        nc.vector.tensor_copy(ot[:], po[:])
        nc.gpsimd.dma_start(out_d[tt * 128:(tt + 1) * 128, :], ot[:])

    # ---- filler queue: dependency-ordered; groups flush a prefix, the
    # deficit counter pulls the rest between attention chunks ----
    fillers = []
    fidx = [0]

    def pull_one():
        if fidx[0] < len(fillers):
            fillers[fidx[0]]()
            fidx[0] += 1
            return True
        return False

    def flush_to(n):
        while fidx[0] < n:
            pull_one()

    # preamble units (emitted directly): deps of G(0,0) scores + first PVs
    pre = [lambda: h_qk(4, 0), lambda: h_qk(0, 0),
           lambda: h_v(0), lambda: h_v(1)]

    def fq(ec, w):
        return lambda: h_qk(ec, w)

    def fv(tt):
        return lambda: h_v(tt)

    # group order is pr-major: G(0,0),G(1,0),G(2,0),G(3,0),G(0,1),...
    # so the ACT-heavy qt=3 groups are spread across the stream; o_proj for
    # q-tile qt runs right after its last group G(qt,3).
    # each group's new deps form a prefix of the remaining filler queue
    gorder = [(qt, pr) for pr in range(4) for qt in range(4)]
    gdep = {}
    for qt, pr in gorder:
        if pr == 0:
            if qt == 0:
                fillers += [fv(2), fv(3)]
            else:
                fillers += [fq(4, qt), fq(0, qt)]
                fillers += [fv(tt) for tt in range(4 * qt, 4 * qt + 4)]
        else:
            fillers += [fq(4 + pr, qt), fq(pr, qt)]
        gdep[(qt, pr)] = len(fillers)

    # ---- attention chunk stream ----
    def s_chunk(qt, pr, kc, sts):
        i = kc - 4 * qt
        lo = max(i, 0) * 128
        sp = ps.tile([128, 1024], f32, tag="s", name=f"sp{qt}_{pr}_{kc}")
        nc.tensor.matmul(
            sp[:, lo:512], qk[0:64, 4 + pr, kc * 128:(kc + 1) * 128],
            qk[0:64, pr, qt * 512 + lo:(qt + 1) * 512],
            start=True, stop=True, tile_position=(0, 0))
        nc.tensor.matmul(
            sp[:, 512 + lo:1024],
            qk[64:128, 4 + pr, kc * 128:(kc + 1) * 128],
            qk[64:128, pr, qt * 512 + lo:(qt + 1) * 512],
            start=True, stop=True, tile_position=(64, 0))
        st = stp.tile([128, 1024], bf16, tag="st", name=f"st{qt}_{pr}_{kc}")
        if "noexp" in opts:
            nc.vector.tensor_copy(st[0:1, 0:2], sp[0:1, 0:2])
            sts[kc] = st
            return
        sp3 = sp[:].rearrange("p (h q) -> p h q", h=2)[:, :, lo:]
        st3 = st[:].rearrange("p (h q) -> p h q", h=2)[:, :, lo:]
        nc.scalar.activation(st3, sp3, Exp, scale=0.125)
        if i >= 0:      # diagonal chunk: mask the triangular blocks
            nc.vector.tensor_mul(
                st[:, lo:lo + 128], st[:, lo:lo + 128], masks[:])
            nc.vector.tensor_mul(
                st[:, 512 + lo:512 + lo + 128],
                st[:, 512 + lo:512 + lo + 128], masks[:])
        sts[kc] = st

    def p_chunk(qt, pr, kc, nkc, pvp, sts):
        lo = max(kc - 4 * qt, 0) * 128
        st = sts[kc]
        nc.tensor.matmul(
            pvp[:, lo:512], vt[:, kc, 2 * pr, :], st[:, lo:512],
            start=(kc == 0), stop=(kc == nkc - 1))
        nc.tensor.matmul(
            pvp[:, 512 + lo:1024], vt[:, kc, 2 * pr + 1, :],
            st[:, 512 + lo:1024],
            start=(kc == 0), stop=(kc == nkc - 1))

    def n_group(qt, pr, pvp):
        # normalize both heads: ob[:, pr, qt] = pv[0:64] * (1/rowsum)
        # (custom-DVE ops ignore the input AP partition base, so stage the
        # sums row at partition 0 first)
        sd = norm.tile([1, 1024], f32, tag="sd")
        nc.vector.reciprocal_approx_fast(sd[:], pvp[0:1, :])
        rb = norm.tile([64, 1024], f32, tag="rb")
        nc.gpsimd.partition_broadcast(rb[:], sd[:])
        for hh in range(2):
            nc.vector.tensor_mul(
                ob[64 * hh:64 * hh + 64, pr, qt * 512:(qt + 1) * 512],
                pvp[1:65, hh * 512:(hh + 1) * 512],
                rb[:, hh * 512:(hh + 1) * 512])

    # modeled per-chunk engine costs (ns) drive filler injection
    FILL_NS = 1800.0        # one 8-matmul half-unit on PE

    deficit = [0.0]

    def maybe_fill():
        while deficit[0] >= FILL_NS:
            if not pull_one():
                return
            deficit[0] -= FILL_NS

    for fn in pre:
        fn()

    if "noattn" in opts:
        flush_to(len(fillers))
        nc.gpsimd.memset(ob[:], 0.5)
        for qt in range(4):
            for tt in range(4 * qt, 4 * qt + 4):
                h_o(qt, tt)
        return
    for qt, pr in gorder:
        flush_to(gdep[(qt, pr)])
        nkc = 4 * qt + 4
        pvp = ps.tile([65, 1024], f32, tag="pv", name=f"pvp{qt}_{pr}")
        sts = {}
        for kc in range(nkc):
            lo = max(kc - 4 * qt, 0) * 128
            s_chunk(qt, pr, kc, sts)
            if kc > 0:
                p_chunk(qt, pr, kc - 1, nkc, pvp, sts)
            deficit[0] += ((1024 - 2 * lo + 352) / 1.2
                           - (3 * (512 - lo) / 2.4 + 120.0))
            maybe_fill()
        # cover exp(last) latency with one filler before the final PV
        pull_one()
        p_chunk(qt, pr, nkc - 1, nkc, pvp, sts)
        n_group(qt, pr, pvp)
        if pr == 3:
            pull_one()  # cover the normalize chain before o_proj reads ob
            for tt in range(4 * qt, 4 * qt + 4):
                h_o(qt, tt)
    flush_to(len(fillers))


def _build(reps=1, opts=()):
    import concourse.mybir as mybir
    import concourse.tile as tile
    from concourse import bacc

    bf16 = mybir.dt.bfloat16
    f32 = mybir.dt.float32

    nc = bacc.Bacc("TRN2", target_bir_lowering=False, debug=False,
                   num_devices=NCORES)
    xt_d = nc.dram_tensor("xt", [8, 128, T], bf16, kind="ExternalInput")
    wq_d = nc.dram_tensor("wq", [8, 128, 1536], bf16, kind="ExternalInput")
    wo_d = nc.dram_tensor("wo", [4, 128, D], bf16, kind="ExternalInput")
    out_d = nc.dram_tensor("out", [T, D], bf16, kind="ExternalOutput")

    with tile.TileContext(nc) as tc:
        with (
            tc.tile_pool(name="cst", bufs=1) as cst,
            tc.tile_pool(name="big", bufs=1) as big,
            tc.tile_pool(name="work", bufs=2) as work,
            tc.tile_pool(name="norm", bufs=2) as norm,
            tc.tile_pool(name="stp", bufs=6) as stp,
            tc.tile_pool(name="ps", bufs=2, space="PSUM") as ps,
        ):
            # static causal mask for the 128x128 diagonal blocks:
            # masks[p, q] = 1 if q >= p else 0
            masks = cst.tile([128, 128], bf16)
            nc.gpsimd.memset(masks[:], 1.0)
            nc.gpsimd.affine_select(
                out=masks[:], in_=masks[:],
                compare_op=mybir.AluOpType.is_ge, fill=0.0,
                base=0, channel_multiplier=-1, pattern=[[1, 128]],
            )
            pools = (cst, big, work, norm, stp, ps)
            dram = (xt_d, wq_d, wo_d, out_d, masks)
            if reps == 1:
                _emit(nc, tc, pools, dram, opts)
            else:
                with tc.For_i(0, reps, 1):
                    _emit(nc, tc, pools, dram, opts)

    nc.compile()
    return nc


def prep_inputs(x, w_qkv, w_o):
    """Host-side shard + layout prep. Returns in_maps for cores 0..7."""
    bf = ml_dtypes.bfloat16
    in_maps = []
    for c in range(NCORES):
        b, hh = c // 2, HPC * (c % 2)
        qrows = w_qkv[hh * DK:(hh + HPC) * DK]                    # [512, 1024]
        krows = w_qkv[D + hh * DK:D + (hh + HPC) * DK]
        vrows = w_qkv[2 * D + hh * DK:2 * D + (hh + HPC) * DK]
        wqt = np.concatenate([qrows, krows, vrows], 0).T          # [1024, 1536]
        in_maps.append({
            "xt": np.ascontiguousarray(x[b].T).astype(bf).reshape(8, 128, T),
            "wq": wqt.astype(bf).reshape(8, 128, 1536),
            "wo": np.ascontiguousarray(w_o[:, hh * DK:(hh + HPC) * DK].T)
                    .astype(bf).reshape(4, 128, D),
        })
    return in_maps


def get_nc(reps=1, opts=()):
    key = ("nc", reps, tuple(opts))
    if key not in _cache:
        _cache[key] = _build(reps, tuple(opts))
    return _cache[key]


def kernel(x, w_qkv, w_o):
    from concourse.bass_utils import run_bass_kernel_spmd

    nc = get_nc()
    in_maps = prep_inputs(np.asarray(x, dtype=np.float32),
                          np.asarray(w_qkv, dtype=np.float32),
                          np.asarray(w_o, dtype=np.float32))
    try:
        res = run_bass_kernel_spmd(nc, in_maps, core_ids=list(range(NCORES)))
    except Exception:
        # transient device faults (e.g. NRT_EXEC_UNIT_UNRECOVERABLE) have
        # been observed once on an otherwise-correct build; retry once
        res = run_bass_kernel_spmd(nc, in_maps, core_ids=list(range(NCORES)))
    out = np.empty((B, T, D), np.float32)
    for b in range(B):
        out[b] = (res.results[2 * b]["out"].astype(np.float32)
                  + res.results[2 * b + 1]["out"].astype(np.float32))
    return out
